# revision 16
# baseline (speedup 1.0000x reference)
"""Cross multi-head attention + residual + LayerNorm on 8 Trainium2 NeuronCores.

Reference (per batch b):
    q = x_q @ Wq.T + bq ; k = x_kv @ Wk.T + bk ; v = x_kv @ Wv.T + bv
    per head: ctx = softmax(q k^T / sqrt(64)) v
    out = concat(ctx) @ Wo.T + bo ;  y = LayerNorm(out + x_q) * gamma + beta

Sharding (8 cores, ZERO inter-core communication): data parallel on batch
(2 groups of 4 cores), query-band parallel within a group (each core owns a
512-row query band). Every core computes K/V for ALL heads over the full kv
sequence (replicated), attention for all 16 heads over its 512 queries, the
full output projection for its rows, and residual + LayerNorm locally.

Input marshalling (host side, in kernel()): activations are transposed to
[e, token] layout and cast to fp8e4m3, and the four weight matrices are
pre-transposed/cast to fp8e4m3, so the device program runs projections
directly with no PE transposes and no on-device weight conversion.

Precision: all matmuls in fp8 with DoubleRow perf mode (2 K-tiles per
pass); operands are fp8e4m3 except the exp'd scores, which use fp8e5m2
(range up to 57344) because scores reach ~9 sigma and would overflow e4m3.
Softmax therefore skips max-subtraction entirely. The softmax denominator
comes from an extra all-ones column appended to V, so the context matmul
emits [ctx; den] in one PSUM pass. Error budget: attention output is ~3.6%
of the residual stream, so ~5% relative error inside attention lands ~2e-3
on the final output.

Self-contained: hardcodes shapes for B=2, L=2048, E=1024, H=16, Dh=64.
"""

from contextlib import ExitStack

import ml_dtypes
import numpy as np

import concourse.bass as bass
import concourse.mybir as mybir
import concourse.tile as tile
from concourse.bass_test_utils import run_kernel

F32 = mybir.dt.float32
FP8 = mybir.dt.float8e4
FP8E5 = mybir.dt.float8e5
DR = mybir.MatmulPerfMode.DoubleRow
NP8 = ml_dtypes.float8_e4m3

B = 2
L = 2048          # kv sequence length
LQ = 512          # query rows per core
E = 1024          # embed
H = 16            # heads
EC = E // 128     # 8 e-chunks
JC = L // 128     # 16 kv chunks of 128
LN_EPS = 1e-5


def make_attention_kernel(iters=1):
    def _k(tc, outs, ins):
        if iters == 1:
            _body(tc, outs, ins)
        else:
            with tc.For_i(0, iters):
                _body(tc, outs, ins)
    return _k


def _body(tc: tile.TileContext, outs, ins):
    nc = tc.nc
    (out,) = outs            # [4, 128, 1024] the core's 512 output rows
    (xq, xqT8d, xkvT8d, wq8d, wk8d, wv8d, wo8d,
     bqc, bkc, bv, bo, gamma, beta) = ins
    # xq:    [512, 1024] f32 residual rows
    # xqT8d: [128, 8, 512]  fp8 x_q^T   (p=e%128, c=e//128, i)
    # xkvT8d:[128, 8, 2048] fp8 x_kv^T  (p=e%128, c=e//128, j)
    # wq8d/wk8d/wv8d: [128, 8, 1024] fp8 W.T as (p=e%128, c=e//128, d)
    # wo8d:  [128, 8, 1024] fp8 Wo.T as (p=hd%128, c=hd//128, e)
    # bqc/bkc: [128, 8] f32 bias columns per d-chunk
    # bv/bo/gamma/beta: [1024] f32

    ctx = ExitStack()
    singles = ctx.enter_context(tc.tile_pool(name="singles", bufs=1))
    big = ctx.enter_context(tc.tile_pool(name="big", bufs=1))
    ktp = ctx.enter_context(tc.tile_pool(name="ktp", bufs=2))
    ex_pool = ctx.enter_context(tc.tile_pool(name="ex", bufs=4))
    small = ctx.enter_context(tc.tile_pool(name="small", bufs=2))
    evac = ctx.enter_context(tc.tile_pool(name="evac", bufs=2))
    psA = ctx.enter_context(tc.tile_pool(name="psA", bufs=2, space="PSUM"))
    psB = ctx.enter_context(tc.tile_pool(name="psB", bufs=2, space="PSUM"))

    # ---- constants ----------------------------------------------------------
    bqc_sb = singles.tile([128, 8], F32, name="bqc_sb")
    nc.sync.dma_start(out=bqc_sb[:], in_=bqc[:])
    bkc_sb = singles.tile([128, 8], F32, name="bkc_sb")
    nc.sync.dma_start(out=bkc_sb[:], in_=bkc[:])
    bv_bc = singles.tile([128, E], F32, name="bv_bc")
    nc.gpsimd.dma_start(out=bv_bc[:], in_=bv[None, :].to_broadcast([128, E]))
    bo_bc = singles.tile([128, E], F32, name="bo_bc")
    nc.gpsimd.dma_start(out=bo_bc[:], in_=bo[None, :].to_broadcast([128, E]))
    gamma_bc = singles.tile([128, E], F32, name="gamma_bc")
    nc.gpsimd.dma_start(out=gamma_bc[:], in_=gamma[None, :].to_broadcast([128, E]))
    beta_bc = singles.tile([128, E], F32, name="beta_bc")
    nc.gpsimd.dma_start(out=beta_bc[:], in_=beta[None, :].to_broadcast([128, E]))
    eps_sb = singles.tile([128, 1], F32, name="eps_sb")
    nc.vector.memset(eps_sb[:], LN_EPS)

    # ---- persistent tensors -------------------------------------------------
    # kT8dr: partition = 32*(h%4) + dh%32 ; free = [t=dh//32, hg=h//4, j]
    kT8dr = big.tile([128, 2, 4, L], FP8, name="kT8dr")
    qT8dr = big.tile([128, 2, 4, LQ], FP8, name="qT8dr")
    # v8: partition = j%128 ; free = [jc, h, 64+1]; col 64 = ones (denominator)
    v8 = big.tile([128, JC, H, 65], FP8, name="v8")
    nc.vector.memset(v8[:, :, :, 64:65].rearrange("p a b c -> p (a b c)"), 1.0)
    # ctxT8b: partition = hd%128 ; free = [c=hd//128, i]
    ctxT8b = big.tile([128, EC, LQ], FP8, name="ctxT8b")

    wq8 = big.tile([128, EC, E], FP8, name="wq8")
    nc.sync.dma_start(out=wq8[:], in_=wq8d[:])
    wk8 = big.tile([128, EC, E], FP8, name="wk8")
    nc.sync.dma_start(out=wk8[:], in_=wk8d[:])
    wv8 = big.tile([128, EC, E], FP8, name="wv8")
    nc.sync.dma_start(out=wv8[:], in_=wv8d[:])
    wo8T = big.tile([128, EC, E], FP8, name="wo8T")
    nc.sync.dma_start(out=wo8T[:], in_=wo8d[:])
    xqT8 = big.tile([128, EC, LQ], FP8, name="xqT8")
    nc.sync.dma_start(out=xqT8[:], in_=xqT8d[:])
    xkvT8 = big.tile([128, EC, L], FP8, name="xkvT8")
    nc.sync.dma_start(out=xkvT8[:], in_=xkvT8d[:])
    # residual rows
    xq_sb = big.tile([128, 4, E], F32, name="xq_sb")
    nc.sync.dma_start(out=xq_sb[:], in_=xq.rearrange("(a p) e -> p a e", p=128))

    # ---- q projection (fp8 DR), evac, shuffle -------------------------------
    qT8 = ktp.tile([128, EC, LQ], FP8, name="qT8", tag="qT8")
    for c in range(EC):
        pq = psB.tile([128, LQ], F32, name=f"pq_{c}", tag="psB")
        for ep in range(4):
            nc.tensor.matmul(
                pq[:],
                wq8[:, 2 * ep : 2 * ep + 2, c * 128 : (c + 1) * 128],
                xqT8[:, 2 * ep : 2 * ep + 2, :],
                start=(ep == 0),
                stop=(ep == 3),
                perf_mode=DR,
            )
        nc.vector.tensor_scalar(
            out=qT8[:, c, :], in0=pq[:], scalar1=bqc_sb[:, c : c + 1],
            scalar2=None, op0=mybir.AluOpType.add,
        )
    # shuffle qT8 [128, c, i] -> qT8dr [32m+p, t, hg, i]
    for m in range(4):
        for t in range(2):
            nc.sync.dma_start(
                out=qT8dr[32 * m : 32 * (m + 1), t, :, :],
                in_=qT8.rearrange("p (g c2) i -> p g c2 i", c2=2)[
                    64 * (m % 2) + 32 * t : 64 * (m % 2) + 32 * t + 32,
                    :, m // 2, :],
            )

    # ---- K/V projections per 512-row kv tile --------------------------------
    for jt in range(4):
        kT8 = ktp.tile([128, EC, 512], FP8, name=f"kT8_{jt}", tag="kT8")
        for c in range(EC):
            pk = psB.tile([128, 512], F32, name=f"pk_{jt}_{c}", tag="psB")
            for ep in range(4):
                nc.tensor.matmul(
                    pk[:],
                    wk8[:, 2 * ep : 2 * ep + 2, c * 128 : (c + 1) * 128],
                    xkvT8[:, 2 * ep : 2 * ep + 2, jt * 512 : (jt + 1) * 512],
                    start=(ep == 0),
                    stop=(ep == 3),
                    perf_mode=DR,
                )
            nc.vector.tensor_scalar(
                out=kT8[:, c, :], in0=pk[:], scalar1=bkc_sb[:, c : c + 1],
                scalar2=None, op0=mybir.AluOpType.add,
            )
        # shuffle into kT8dr
        for m in range(4):
            for t in range(2):
                nc.sync.dma_start(
                    out=kT8dr[32 * m : 32 * (m + 1), t, :,
                              jt * 512 : (jt + 1) * 512],
                    in_=kT8.rearrange("p (g c2) j -> p g c2 j", c2=2)[
                        64 * (m % 2) + 32 * t : 64 * (m % 2) + 32 * t + 32,
                        :, m // 2, :],
                )
        # V projection per j-chunk (psB banks so attention's psA is free)
        for jj in range(4):
            jc = jt * 4 + jj
            for half in range(2):
                pv = psB.tile([128, 512], F32, name=f"pv_{jc}_{half}",
                              tag="psB")
                for ep in range(4):
                    nc.tensor.matmul(
                        pv[:],
                        xkvT8[:, 2 * ep : 2 * ep + 2,
                              jc * 128 : (jc + 1) * 128],
                        wv8[:, 2 * ep : 2 * ep + 2,
                            half * 512 : (half + 1) * 512],
                        start=(ep == 0),
                        stop=(ep == 3),
                        perf_mode=DR,
                    )
                nc.vector.tensor_tensor(
                    out=v8[:, jc, 8 * half : 8 * (half + 1), 0:64],
                    in0=pv.rearrange("p (h d) -> p h d", d=64),
                    in1=bv_bc.rearrange("p (h d) -> p h d", d=64)[
                        :, 8 * half : 8 * (half + 1), :],
                    op=mybir.AluOpType.add,
                )

    # ---- attention: per head, accumulate ctx over jc pairs ------------------
    for h in range(H):
        m, hg = h % 4, h // 4
        pc = psB.tile([65, 512], F32, name=f"pc_{h}", tag="psC")
        for jp in range(8):
            s_ps = psA.tile([128, 2, 512], F32, name=f"sps_{h}_{jp}", tag="psA")
            for u in range(2):
                jc = 2 * jp + u
                nc.tensor.matmul(
                    s_ps[:, u, :],
                    kT8dr[32 * m : 32 * (m + 1), :, hg,
                          jc * 128 : (jc + 1) * 128],
                    qT8dr[32 * m : 32 * (m + 1), :, hg, :],
                    start=True,
                    stop=True,
                    perf_mode=DR,
                    tile_position=(32 * m, 0),
                )
            ex = ex_pool.tile([128, 2, 512], FP8E5, name=f"ex_{h}_{jp}",
                              tag="ex")
            nc.scalar.activation(
                out=ex[:],
                in_=s_ps[:],
                func=mybir.ActivationFunctionType.Exp,
                scale=0.125,
            )
            nc.tensor.matmul(
                pc[:],
                v8[:, 2 * jp : 2 * jp + 2, h, :],
                ex[:],
                start=(jp == 0),
                stop=(jp == 7),
                perf_mode=DR,
            )
        # normalize: rows 0-63 = ctx^T, row 64 = denominator
        den = small.tile([1, 512], F32, name=f"den_{h}", tag="den")
        nc.vector.reciprocal(den[:], pc[64:65, :])
        bc = small.tile([64, 512], F32, name=f"bc_{h}", tag="bc")
        nc.gpsimd.partition_broadcast(bc[:], den[:], channels=64)
        ctx64 = small.tile([64, 512], FP8, name=f"ctx64_{h}", tag="ctx64")
        nc.vector.tensor_tensor(
            out=ctx64[:], in0=pc[0:64, :], in1=bc[:],
            op=mybir.AluOpType.mult,
        )
        nc.sync.dma_start(
            out=ctxT8b[64 * (h % 2) : 64 * (h % 2) + 64, h // 2, :],
            in_=ctx64[:],
        )

    # ---- output projection + residual + LayerNorm per 128-row block ---------
    for ib in range(4):
        po = psA.tile([128, 2, 512], F32, name=f"po_{ib}", tag="psA")
        for half in range(2):
            for u in range(4):
                nc.tensor.matmul(
                    po[:, half, :],
                    ctxT8b[:, 2 * u : 2 * u + 2, ib * 128 : (ib + 1) * 128],
                    wo8T[:, 2 * u : 2 * u + 2, half * 512 : (half + 1) * 512],
                    start=(u == 0),
                    stop=(u == 3),
                    perf_mode=DR,
                )
        xt = evac.tile([128, E], F32, name=f"xt_{ib}", tag=f"nat{ib % 2}")
        nc.vector.scalar_tensor_tensor(
            out=xt[:], in0=po.rearrange("p a b -> p (a b)"), scalar=1.0,
            in1=xq_sb[:, ib, :],
            op0=mybir.AluOpType.mult, op1=mybir.AluOpType.add,
        )
        nc.gpsimd.tensor_tensor(out=xt[:], in0=xt[:], in1=bo_bc[:],
                                op=mybir.AluOpType.add)
        stats = small.tile([128, 2, 6], F32, name=f"st_{ib}", tag="st")
        for hh in range(2):
            nc.vector.bn_stats(out=stats[:, hh, :],
                               in_=xt[:, hh * 512 : (hh + 1) * 512])
        mv = small.tile([128, 2], F32, name=f"mv_{ib}", tag="mv")
        nc.vector.bn_aggr(out=mv[:], in_=stats.rearrange("p a b -> p (a b)"))
        rstd = small.tile([128, 1], F32, name=f"rstd_{ib}", tag="rstd")
        nc.scalar.activation(
            out=rstd[:],
            in_=mv[:, 1:2],
            func=mybir.ActivationFunctionType.Sqrt,
            bias=eps_sb[:],
        )
        nc.vector.reciprocal(rstd[:], rstd[:])
        nc.vector.tensor_scalar(
            out=xt[:],
            in0=xt[:],
            scalar1=mv[:, 0:1],
            scalar2=rstd[:],
            op0=mybir.AluOpType.subtract,
            op1=mybir.AluOpType.mult,
        )
        nc.vector.tensor_tensor(out=xt[:], in0=xt[:], in1=gamma_bc[:],
                                op=mybir.AluOpType.mult)
        nc.gpsimd.tensor_tensor(out=xt[:], in0=xt[:], in1=beta_bc[:],
                                op=mybir.AluOpType.add)
        nc.sync.dma_start(out=out[ib], in_=xt[:])

    ctx.close()


def _to_pce(mat):
    """[E, N] -> [128, E//128, N] (p = e%128, c = e//128) cast to fp8."""
    return np.ascontiguousarray(
        mat.reshape(EC, 128, mat.shape[1]).transpose(1, 0, 2).astype(NP8))


def _prepare_inputs(query_seq, key_value_seq, Wq, bq, Wk, bk, Wv, bv, Wo, bo,
                    ln_gamma, ln_beta):
    """Build the 8 per-core input tuples (host-side layout + fp8 cast)."""
    wq8 = _to_pce(np.ascontiguousarray(Wq.T))
    wk8 = _to_pce(np.ascontiguousarray(Wk.T))
    wv8 = _to_pce(np.ascontiguousarray(Wv.T))
    wo8 = _to_pce(np.ascontiguousarray(Wo.T))
    bqc = np.ascontiguousarray(bq.reshape(8, 128).T)
    bkc = np.ascontiguousarray(bk.reshape(8, 128).T)
    ins = []
    for c in range(8):
        b, r = divmod(c, 4)
        xq = np.ascontiguousarray(query_seq[b, 512 * r : 512 * (r + 1)])
        xqT8 = _to_pce(np.ascontiguousarray(xq.T))
        xkvT8 = _to_pce(np.ascontiguousarray(key_value_seq[b].T))
        ins.append((xq, xqT8, xkvT8, wq8, wk8, wv8, wo8, bqc, bkc,
                    np.ascontiguousarray(bv), np.ascontiguousarray(bo),
                    np.ascontiguousarray(ln_gamma),
                    np.ascontiguousarray(ln_beta)))
    return ins


def kernel(**inputs) -> np.ndarray:
    query_seq = np.asarray(inputs["query_seq"], dtype=np.float32)
    key_value_seq = np.asarray(inputs["key_value_seq"], dtype=np.float32)
    args = {
        k: np.asarray(inputs[k], dtype=np.float32)
        for k in ("Wq", "bq", "Wk", "bk", "Wv", "bv", "Wo", "bo",
                  "ln_gamma", "ln_beta")
    }
    ins = _prepare_inputs(query_seq, key_value_seq, **args)
    out_like = [(np.zeros((4, 128, E), np.float32),) for _ in range(8)]
    res = run_kernel(
        make_attention_kernel(1),
        None,
        ins,
        bass_type=tile.TileContext,
        num_cores=8,
        check_with_sim=False,
        check_with_hw=True,
        trace_sim=False,
        output_like=out_like,
    )
    out = np.empty((B, L, E), np.float32)
    for c in range(8):
        bnd = res.results[c]["0_dram"]  # [4, 128, 1024]
        b, r = divmod(c, 4)
        out[b, 512 * r : 512 * (r + 1), :] = bnd.reshape(512, E)
    return out
